# revision 26
# baseline (speedup 1.0000x reference)
"""Trainium2 Bass kernel for nn_DynamicFeedForward (dynamic conv3d + BN + ReLU).

Strategy: data-parallel over the batch dim (8 samples -> 8 NeuronCores).
The tiny gating MLP + per-sample kernel synthesis (<0.1% of FLOPs) runs on
host in fp32; each core runs its sample's 3x3x3 conv as 27 shifted matmuls
(bf16 inputs, fp32 PSUM accumulation), computes per-core BN partial sums,
all-reduces them across the 8 cores, and applies the fused affine+ReLU.
"""
import sys

sys.path.insert(0, "/opt/trn_rl_repo")

from contextlib import ExitStack

import ml_dtypes
import numpy as np

import concourse.bacc as bacc
import concourse.tile as tile
from concourse import mybir
from concourse.bass_utils import run_bass_kernel_spmd

# problem dims (hardcoded per contract)
B, C_IN, DIMS, NW = 8, 64, 64, 8
T, H, W = 16, 56, 56
HID = C_IN // 4 + 1
EPS = 1e-5
N_CORES = 8

# device tiling
SL = H * W             # slab elements per t-slice per partition (contiguous)
HB = 8                 # output h-rows per position tile
NPT = HB * W           # matmul free size (448)
TPT = H // HB          # 7 position tiles per t-slice
NPAIR = T * TPT // 2   # 56 pairs of position tiles (col0/col64)
RING = 4
TOT = float(B * T * H * W)   # BN element count per channel
HALF = float(T * H * W // 2) # elements per partition-half per core

f32 = mybir.dt.float32
bf16 = mybir.dt.bfloat16

# all 27 taps split across the two PE row groups (both slab halves hold the
# same x slice; w-edges handled by trimming the matmul output window).
# Each group leads with full-coverage (dh=1, dw=1) taps so that, after
# t-boundary filtering, its first matmul always covers the whole PSUM tile
# (required for the start_tensor_calc zero-region protocol).
_FULL = [(1, 1, 1), (0, 1, 1), (2, 1, 1)]
_REST = [(dt, dh, dw) for dt in range(3) for dh in range(3) for dw in range(3)
         if (dt, dh, dw) not in _FULL]
A_TAPS = [_FULL[0]] + _REST[:13]
B_TAPS = _FULL[1:] + _REST[13:]
NSLOT_A = len(A_TAPS)  # 14
NSLOT_B = len(B_TAPS)  # 13


_KEEP_LDW = False
_USE_DEP_HELPERS = True
_EXPLICIT_LDW = False


def _build(for_sim=False, no_tail=False, max_pairs=None, dbg=0):
    nc = bacc.Bacc("TRN2", target_bir_lowering=False, debug=False,
                   num_devices=1 if for_sim else N_CORES)
    x_d = nc.dram_tensor("xbf", [C_IN, T, H, W], bf16, kind="ExternalInput").ap()
    w_d = nc.dram_tensor("wpk", [128, NSLOT_A * 64], bf16, kind="ExternalInput").ap()
    db_d = nc.dram_tensor("dynb", [128, 1], f32, kind="ExternalInput").ap()
    gb_d = nc.dram_tensor("gb", [64, 2], f32, kind="ExternalInput").ap()
    out_d = nc.dram_tensor("out", [DIMS, T, H, W], f32, kind="ExternalOutput").ap()
    ydbg_d = None
    if for_sim:
        ydbg_d = nc.dram_tensor("ydbg", [128, NPAIR * NPT], f32, kind="ExternalOutput").ap()

    with tile.TileContext(nc) as tc, ExitStack() as ctx:
        per = ctx.enter_context(tc.tile_pool(name="per", bufs=1))
        sm = ctx.enter_context(tc.tile_pool(name="sm", bufs=2))
        ps = ctx.enter_context(tc.tile_pool(name="ps", bufs=2, space="PSUM"))
        dram = ctx.enter_context(tc.tile_pool(name="dram", bufs=1, space="DRAM"))

        wt = per.tile([128, NSLOT_A * 64], bf16, tag="wt")
        dbt = per.tile([128, 1], f32, tag="dbt")
        gbt = per.tile([64, 2], f32, tag="gbt")
        nc.sync.dma_start(wt[:], w_d[:])
        nc.sync.dma_start(dbt[:], db_d[:])
        nc.sync.dma_start(gbt[:], gb_d[:])

        yt = per.tile([128, NPAIR * NPT], f32, tag="yt")  # 25088 cols
        statb = per.tile([128, NPAIR * 6], f32, tag="statb")

        slabs = [per.tile([128, SL], bf16, tag=f"slab{i}", name=f"slab{i}") for i in range(RING)]
        def dma_slice(t):
            s = slabs[t % RING]
            xv = x_d[:, t, :, :].rearrange("p h w -> p (h w)")
            nc.sync.dma_start(s[0:64, :], xv)
            nc.sync.dma_start(s[64:128, :], xv)

        dma_slice(0)
        dma_slice(1)
        dma_slice(2)

        def tap_lists(t):
            la = [(i, tp) for i, tp in enumerate(A_TAPS) if 0 <= t + tp[0] - 1 < T]
            lb = [(i, tp) for i, tp in enumerate(B_TAPS) if 0 <= t + tp[0] - 1 < T]
            return la, lb

        cur_t = 0
        for m in range(max_pairs if max_pairs else NPAIR):
            gs = (2 * m, 2 * m + 1)
            t_a = gs[0] // TPT
            if t_a > cur_t:
                cur_t = t_a
                if cur_t + 2 < T:
                    dma_slice(cur_t + 2)
            pst00 = ps.tile([128, NPT], f32, tag="pst00", name="pst00")
            pst10 = ps.tile([128, NPT], f32, tag="pst10", name="pst10")
            pst01 = ps.tile([128, NPT], f32, tag="pst01", name="pst01")
            pst11 = ps.tile([128, NPT], f32, tag="pst11", name="pst11")
            psts = {(True, 0): pst00, (False, 0): pst10,
                    (True, 1): pst01, (False, 1): pst11}
            metas = []
            for colh, g in enumerate(gs):
                t_g, i_g = g // TPT, g % TPT
                h0 = i_g * HB
                la, lb = tap_lists(t_g)
                metas.append((t_g, h0, la, lb))
            nslot = max(max(len(mt[2]), len(mt[3])) for mt in metas)
            for s in range(nslot):
                for isA, rowb in ((True, 0), (False, 64)):
                    for colh in range(2):
                        t_g, h0, la, lb = metas[colh]
                        lst = la if isA else lb
                        if s >= len(lst):
                            continue
                        widx, (dt, dh, dw) = lst[s]
                        si = (t_g + dt - 1) % RING
                        lhsT = wt[rowb:rowb + 64, widx * 64:(widx + 1) * 64]
                        r_lo = max(0, 1 - dh - h0)
                        r_hi = min(HB, H - h0 - dh + 1)
                        ihlo = h0 + r_lo + dh - 1
                        rows = r_hi - r_lo
                        w_lo = max(0, 1 - dw)
                        w_hi = min(W, W + 1 - dw)
                        sv = slabs[si][rowb:rowb + 64, :].rearrange(
                            "p (h w) -> p h w", h=H, w=W)
                        rhs = sv[:, ihlo:ihlo + rows, w_lo + dw - 1:w_hi + dw - 1]
                        ov = psts[(isA, colh)][colh * 64:(colh + 1) * 64, :].rearrange(
                            "p (r w) -> p r w", r=HB, w=W)
                        outap = ov[:, r_lo:r_hi, w_lo:w_hi]
                        first = (s == 0)
                        lastmm = (s == len(lst) - 1)
                        if dbg == 1:
                            continue
                        if for_sim and rows > 1 and (w_hi - w_lo) < W:
                            for rr in range(rows):
                                nc.tensor.matmul(
                                    outap[:, rr:rr + 1, :], lhsT,
                                    rhs[:, rr:rr + 1, :],
                                    start=first and rr == 0,
                                    stop=lastmm and rr == rows - 1)
                        else:
                            nc.tensor.matmul(outap, lhsT, rhs,
                                             start=first, stop=lastmm)
            k = m
            ysl = yt[:, m * NPT:(m + 1) * NPT]
            # drain with dyn bias add
            if dbg not in (1, 2):
                nc.scalar.activation(ysl[0:64, :], pst00[0:64, :],
                                     mybir.ActivationFunctionType.Identity,
                                     bias=dbt[0:64, 0:1])
                nc.vector.tensor_add(ysl[0:64, :], ysl[0:64, :], pst10[0:64, :])
                nc.scalar.activation(ysl[64:128, :], pst01[64:128, :],
                                     mybir.ActivationFunctionType.Identity,
                                     bias=dbt[64:128, 0:1])
                nc.vector.tensor_add(ysl[64:128, :], ysl[64:128, :], pst11[64:128, :])
                if dbg != 3:
                    nc.vector.bn_stats(statb[:, m * 6:(m + 1) * 6], ysl)

        if for_sim:
            nc.sync.dma_start(ydbg_d[:], yt[:])
        if no_tail:
            for m in range([] if dbg in (1, 2) else range(max_pairs if max_pairs else NPAIR) and 0 for _ in ()) if False else (range(0) if dbg in (1, 2) else range(max_pairs if max_pairs else NPAIR)):
                yv = yt[:, m * NPT:(m + 1) * NPT]
                for colh in range(2):
                    g = 2 * m + colh
                    t_g, i_g = g // TPT, g % TPT
                    h0 = i_g * HB
                    src2 = yv[colh * 64:(colh + 1) * 64, :].rearrange(
                        "p (r w) -> p r w", r=HB, w=W)
                    nc.sync.dma_start(out_d[:, t_g, h0:h0 + HB, :], src2[:])
            nc.compile()
            return nc
        # aggregate stats -> (mean, var) per partition
        mv = sm.tile([128, 2], f32, tag="mv")
        nc.vector.bn_aggr(mv[:], statb[:])
        # convert to (sum, sumsq)
        sq = sm.tile([128, 2], f32, tag="sq")
        # sq col0 = mean * HALF ; col1 = (var + mean^2) * HALF
        m2 = sm.tile([128, 1], f32, tag="m2")
        nc.vector.tensor_mul(m2[:], mv[:, 0:1], mv[:, 0:1])
        nc.vector.tensor_add(sq[:, 1:2], mv[:, 1:2], m2[:])
        nc.vector.tensor_scalar_mul(sq[:, 1:2], sq[:, 1:2], HALF)
        nc.vector.tensor_scalar_mul(sq[:, 0:1], mv[:, 0:1], HALF)
        # fold partition halves: low64 += high64 (cross-partition via DMA)
        fold = sm.tile([64, 2], f32, tag="fold")
        nc.sync.dma_start(fold[:], sq[64:128, :])
        loc = sm.tile([64, 2], f32, tag="loc")
        nc.vector.tensor_add(loc[:], sq[0:64, :], fold[:])
        # all-reduce across cores
        cin = dram.tile([64, 2], f32)
        cout = dram.tile([64, 2], f32)
        if for_sim:
            # emulate 8 identical cores so single-core sim matches host math
            nc.vector.tensor_scalar_mul(loc[:], loc[:], float(N_CORES))
        nc.sync.dma_start(cin[:], loc[:])
        if for_sim:
            nc.sync.dma_start(cout[:], cin[:])
        else:
            nc.gpsimd.collective_compute(
                "AllReduce", mybir.AluOpType.add,
                replica_groups=[list(range(N_CORES))],
                ins=[cin.opt()], outs=[cout.opt()],
            )
        g = sm.tile([64, 2], f32, tag="g")
        nc.sync.dma_start(g[:], cout[:])
        # mean/var -> a = gamma/sqrt(var+eps), b = beta - mean*a
        mean = sm.tile([64, 1], f32, tag="mean")
        e2 = sm.tile([64, 1], f32, tag="e2")
        nc.vector.tensor_scalar_mul(mean[:], g[:, 0:1], 1.0 / TOT)
        nc.vector.tensor_scalar_mul(e2[:], g[:, 1:2], 1.0 / TOT)
        msq = sm.tile([64, 1], f32, tag="msq")
        nc.vector.tensor_mul(msq[:], mean[:], mean[:])
        vpe = sm.tile([64, 1], f32, tag="vpe")
        nc.vector.tensor_sub(vpe[:], e2[:], msq[:])
        nc.vector.tensor_scalar_add(vpe[:], vpe[:], EPS)
        sqv = sm.tile([64, 1], f32, tag="sqv")
        nc.scalar.sqrt(sqv[:], vpe[:])
        inv = sm.tile([64, 1], f32, tag="inv")
        nc.vector.reciprocal(inv[:], sqv[:])
        # one Newton step: inv *= 1.5 - 0.5*vpe*inv*inv
        t1 = sm.tile([64, 1], f32, tag="t1")
        nc.vector.tensor_mul(t1[:], inv[:], inv[:])
        nc.vector.tensor_mul(t1[:], t1[:], vpe[:])
        nc.vector.tensor_scalar(t1[:], t1[:], -0.5, 1.5,
                                mybir.AluOpType.mult, mybir.AluOpType.add)
        nc.vector.tensor_mul(inv[:], inv[:], t1[:])
        ab = sm.tile([128, 2], f32, tag="ab")
        nc.vector.tensor_mul(ab[0:64, 0:1], inv[:], gbt[:, 0:1])
        t2 = sm.tile([64, 1], f32, tag="t2")
        nc.vector.tensor_mul(t2[:], mean[:], ab[0:64, 0:1])
        nc.vector.tensor_sub(ab[0:64, 1:2], gbt[:, 1:2], t2[:])
        nc.sync.dma_start(ab[64:128, :], ab[0:64, :])

        # normalize + relu in chunks, then stream out
        PCH = 7            # pairs per chunk
        CHUNK = PCH * NPT  # 3136
        for c in range(NPAIR // PCH):
            ysl = yt[:, c * CHUNK:(c + 1) * CHUNK]
            nc.scalar.activation(ysl, ysl, mybir.ActivationFunctionType.Relu,
                                 bias=ab[:, 1:2], scale=ab[:, 0:1])
            for m in range(c * PCH, (c + 1) * PCH):
                yv = yt[:, m * NPT:(m + 1) * NPT]
                for colh in range(2):
                    g = 2 * m + colh
                    t_g, i_g = g // TPT, g % TPT
                    h0 = i_g * HB
                    src2 = yv[colh * 64:(colh + 1) * 64, :].rearrange(
                        "p (r w) -> p r w", r=HB, w=W)
                    nc.sync.dma_start(out_d[:, t_g, h0:h0 + HB, :], src2[:])
    nc.compile()
    return nc


_NC = None
_last_in_maps = None


def _get_nc():
    global _NC
    if _NC is None:
        _NC = _build()
    return _NC


def kernel(x, weights, biases, W1, b1, W2, b2, gamma, beta, epochs_num):
    x = np.asarray(x, np.float32)
    weights = np.asarray(weights, np.float32)
    biases = np.asarray(biases, np.float32)
    W1 = np.asarray(W1, np.float32)
    b1 = np.asarray(b1, np.float32)
    W2 = np.asarray(W2, np.float32)
    b2 = np.asarray(b2, np.float32)
    gamma = np.asarray(gamma, np.float32)
    beta = np.asarray(beta, np.float32)
    ep = int(epochs_num)

    # host gating (matches reference math in fp32)
    tau = np.float32(30.0 - 2.9 * ep) if ep < 10 else np.float32(1.0)
    pooled = x.max(axis=(2, 3, 4))                    # (B, C_IN)
    h = np.maximum(pooled @ W1.T + b1, 0.0)
    logits = h @ W2.T + b2
    z = logits / tau
    z = z - z.max(axis=1, keepdims=True)
    ez = np.exp(z)
    phi = (ez / ez.sum(axis=1, keepdims=True)).astype(np.float32)  # (B, NW)

    dyn_w = np.einsum("bn,noidhw->boidhw", phi, weights).astype(np.float32)
    dyn_b = (phi @ biases).astype(np.float32)          # (B, DIMS)

    gb = np.stack([gamma, beta], axis=1).astype(np.float32)

    in_maps = []
    for b in range(B):
        wpk = np.zeros((128, NSLOT_A * 64), np.float32)
        for i, (dt, dh, dw) in enumerate(A_TAPS):
            wpk[0:64, i * 64:(i + 1) * 64] = dyn_w[b, :, :, dt, dh, dw].T
        for i, (dt, dh, dw) in enumerate(B_TAPS):
            wpk[64:128, i * 64:(i + 1) * 64] = dyn_w[b, :, :, dt, dh, dw].T
        in_maps.append({
            "xbf": x[b].astype(ml_dtypes.bfloat16),
            "wpk": wpk.astype(ml_dtypes.bfloat16),
            "dynb": np.tile(dyn_b[b][:, None], (2, 1)).astype(np.float32),
            "gb": gb,
        })

    global _last_in_maps
    _last_in_maps = in_maps
    nc = _get_nc()
    res = run_bass_kernel_spmd(nc, in_maps, core_ids=list(range(N_CORES)))
    out = np.stack([res.results[b]["out"] for b in range(B)], axis=0)
    return out.astype(np.float32)


# revision 27
# speedup vs baseline: 1.0059x; 1.0059x over previous
"""Trainium2 Bass kernel for nn_DynamicFeedForward (dynamic conv3d + BN + ReLU).

Strategy: data-parallel over the batch dim (8 samples -> 8 NeuronCores).
The tiny gating MLP + per-sample kernel synthesis (<0.1% of FLOPs) runs on
host in fp32; each core runs its sample's 3x3x3 conv as 27 shifted matmuls
(bf16 inputs, fp32 PSUM accumulation), computes per-core BN partial sums,
all-reduces them across the 8 cores, and applies the fused affine+ReLU.
"""
import sys

sys.path.insert(0, "/opt/trn_rl_repo")

from contextlib import ExitStack

import ml_dtypes
import numpy as np

import concourse.bacc as bacc
import concourse.tile as tile
from concourse import mybir
from concourse.bass_utils import run_bass_kernel_spmd

# problem dims (hardcoded per contract)
B, C_IN, DIMS, NW = 8, 64, 64, 8
T, H, W = 16, 56, 56
HID = C_IN // 4 + 1
EPS = 1e-5
N_CORES = 8

# device tiling
SL = H * W             # slab elements per t-slice per partition (contiguous)
HB = 8                 # output h-rows per position tile
NPT = HB * W           # matmul free size (448)
TPT = H // HB          # 7 position tiles per t-slice
NPAIR = T * TPT // 2   # 56 pairs of position tiles (col0/col64)
RING = 4
TOT = float(B * T * H * W)   # BN element count per channel
HALF = float(T * H * W // 2) # elements per partition-half per core

f32 = mybir.dt.float32
bf16 = mybir.dt.bfloat16

# all 27 taps split across the two PE row groups (both slab halves hold the
# same x slice; w-edges handled by trimming the matmul output window).
# Each group leads with full-coverage (dh=1, dw=1) taps so that, after
# t-boundary filtering, its first matmul always covers the whole PSUM tile
# (required for the start_tensor_calc zero-region protocol).
_FULL = [(1, 1, 1), (0, 1, 1), (2, 1, 1)]
_REST = [(dt, dh, dw) for dt in range(3) for dh in range(3) for dw in range(3)
         if (dt, dh, dw) not in _FULL]
A_TAPS = [_FULL[0]] + _REST[:13]
B_TAPS = _FULL[1:] + _REST[13:]
NSLOT_A = len(A_TAPS)  # 14
NSLOT_B = len(B_TAPS)  # 13


_KEEP_LDW = False
_USE_DEP_HELPERS = True
_EXPLICIT_LDW = False


def _build(for_sim=False, no_tail=False, max_pairs=None, dbg=0):
    nc = bacc.Bacc("TRN2", target_bir_lowering=False, debug=False,
                   num_devices=1 if for_sim else N_CORES)
    x_d = nc.dram_tensor("xbf", [C_IN, T, H, W], bf16, kind="ExternalInput").ap()
    w_d = nc.dram_tensor("wpk", [128, NSLOT_A * 64], bf16, kind="ExternalInput").ap()
    db_d = nc.dram_tensor("dynb", [128, 1], f32, kind="ExternalInput").ap()
    gb_d = nc.dram_tensor("gb", [64, 2], f32, kind="ExternalInput").ap()
    out_d = nc.dram_tensor("out", [DIMS, T, H, W], f32, kind="ExternalOutput").ap()
    ydbg_d = None
    if for_sim:
        ydbg_d = nc.dram_tensor("ydbg", [128, NPAIR * NPT], f32, kind="ExternalOutput").ap()

    with tile.TileContext(nc) as tc, ExitStack() as ctx:
        per = ctx.enter_context(tc.tile_pool(name="per", bufs=1))
        sm = ctx.enter_context(tc.tile_pool(name="sm", bufs=2))
        ps = ctx.enter_context(tc.tile_pool(name="ps", bufs=2, space="PSUM"))
        dram = ctx.enter_context(tc.tile_pool(name="dram", bufs=1, space="DRAM"))

        wt = per.tile([128, NSLOT_A * 64], bf16, tag="wt")
        dbt = per.tile([128, 1], f32, tag="dbt")
        gbt = per.tile([64, 2], f32, tag="gbt")
        nc.sync.dma_start(wt[:], w_d[:])
        nc.sync.dma_start(dbt[:], db_d[:])
        nc.sync.dma_start(gbt[:], gb_d[:])

        yt = per.tile([128, NPAIR * NPT], f32, tag="yt")  # 25088 cols
        statb = per.tile([128, NPAIR * 6], f32, tag="statb")

        slabs = [per.tile([128, SL], bf16, tag=f"slab{i}", name=f"slab{i}") for i in range(RING)]
        def dma_slice(t):
            s = slabs[t % RING]
            xv = x_d[:, t, :, :].rearrange("p h w -> p (h w)")
            nc.sync.dma_start(s[0:64, :], xv)
            nc.sync.dma_start(s[64:128, :], xv)

        dma_slice(0)
        dma_slice(1)
        dma_slice(2)

        def tap_lists(t):
            la = [(i, tp) for i, tp in enumerate(A_TAPS) if 0 <= t + tp[0] - 1 < T]
            lb = [(i, tp) for i, tp in enumerate(B_TAPS) if 0 <= t + tp[0] - 1 < T]
            return la, lb

        cur_t = 0
        for m in range(max_pairs if max_pairs else NPAIR):
            gs = (2 * m, 2 * m + 1)
            t_a = gs[0] // TPT
            if t_a > cur_t:
                cur_t = t_a
                if cur_t + 2 < T:
                    dma_slice(cur_t + 2)
            pst00 = ps.tile([128, NPT], f32, tag="pst00", name="pst00")
            pst10 = ps.tile([128, NPT], f32, tag="pst10", name="pst10")
            pst01 = ps.tile([128, NPT], f32, tag="pst01", name="pst01")
            pst11 = ps.tile([128, NPT], f32, tag="pst11", name="pst11")
            psts = {(True, 0): pst00, (False, 0): pst10,
                    (True, 1): pst01, (False, 1): pst11}
            metas = []
            for colh, g in enumerate(gs):
                t_g, i_g = g // TPT, g % TPT
                h0 = i_g * HB
                la, lb = tap_lists(t_g)
                metas.append((t_g, h0, la, lb))
            nslot = max(max(len(mt[2]), len(mt[3])) for mt in metas)
            for s in range(nslot):
                for colh in range(2):
                    for isA, rowb in ((True, 0), (False, 64)):
                        t_g, h0, la, lb = metas[colh]
                        lst = la if isA else lb
                        if s >= len(lst):
                            continue
                        widx, (dt, dh, dw) = lst[s]
                        si = (t_g + dt - 1) % RING
                        lhsT = wt[rowb:rowb + 64, widx * 64:(widx + 1) * 64]
                        r_lo = max(0, 1 - dh - h0)
                        r_hi = min(HB, H - h0 - dh + 1)
                        ihlo = h0 + r_lo + dh - 1
                        rows = r_hi - r_lo
                        w_lo = max(0, 1 - dw)
                        w_hi = min(W, W + 1 - dw)
                        sv = slabs[si][rowb:rowb + 64, :].rearrange(
                            "p (h w) -> p h w", h=H, w=W)
                        rhs = sv[:, ihlo:ihlo + rows, w_lo + dw - 1:w_hi + dw - 1]
                        ov = psts[(isA, colh)][colh * 64:(colh + 1) * 64, :].rearrange(
                            "p (r w) -> p r w", r=HB, w=W)
                        outap = ov[:, r_lo:r_hi, w_lo:w_hi]
                        first = (s == 0)
                        lastmm = (s == len(lst) - 1)
                        if dbg == 1:
                            continue
                        if for_sim and rows > 1 and (w_hi - w_lo) < W:
                            for rr in range(rows):
                                nc.tensor.matmul(
                                    outap[:, rr:rr + 1, :], lhsT,
                                    rhs[:, rr:rr + 1, :],
                                    start=first and rr == 0,
                                    stop=lastmm and rr == rows - 1)
                        else:
                            nc.tensor.matmul(outap, lhsT, rhs,
                                             start=first, stop=lastmm)
            k = m
            ysl = yt[:, m * NPT:(m + 1) * NPT]
            # drain with dyn bias add
            if dbg not in (1, 2):
                nc.scalar.activation(ysl[0:64, :], pst00[0:64, :],
                                     mybir.ActivationFunctionType.Identity,
                                     bias=dbt[0:64, 0:1])
                nc.vector.tensor_add(ysl[0:64, :], ysl[0:64, :], pst10[0:64, :])
                nc.scalar.activation(ysl[64:128, :], pst01[64:128, :],
                                     mybir.ActivationFunctionType.Identity,
                                     bias=dbt[64:128, 0:1])
                nc.vector.tensor_add(ysl[64:128, :], ysl[64:128, :], pst11[64:128, :])
                if dbg != 3:
                    nc.vector.bn_stats(statb[:, m * 6:(m + 1) * 6], ysl)

        if for_sim:
            nc.sync.dma_start(ydbg_d[:], yt[:])
        if no_tail:
            for m in range([] if dbg in (1, 2) else range(max_pairs if max_pairs else NPAIR) and 0 for _ in ()) if False else (range(0) if dbg in (1, 2) else range(max_pairs if max_pairs else NPAIR)):
                yv = yt[:, m * NPT:(m + 1) * NPT]
                for colh in range(2):
                    g = 2 * m + colh
                    t_g, i_g = g // TPT, g % TPT
                    h0 = i_g * HB
                    src2 = yv[colh * 64:(colh + 1) * 64, :].rearrange(
                        "p (r w) -> p r w", r=HB, w=W)
                    nc.sync.dma_start(out_d[:, t_g, h0:h0 + HB, :], src2[:])
            nc.compile()
            return nc
        # aggregate stats -> (mean, var) per partition
        mv = sm.tile([128, 2], f32, tag="mv")
        nc.vector.bn_aggr(mv[:], statb[:])
        # convert to (sum, sumsq)
        sq = sm.tile([128, 2], f32, tag="sq")
        # sq col0 = mean * HALF ; col1 = (var + mean^2) * HALF
        m2 = sm.tile([128, 1], f32, tag="m2")
        nc.vector.tensor_mul(m2[:], mv[:, 0:1], mv[:, 0:1])
        nc.vector.tensor_add(sq[:, 1:2], mv[:, 1:2], m2[:])
        nc.vector.tensor_scalar_mul(sq[:, 1:2], sq[:, 1:2], HALF)
        nc.vector.tensor_scalar_mul(sq[:, 0:1], mv[:, 0:1], HALF)
        # fold partition halves: low64 += high64 (cross-partition via DMA)
        fold = sm.tile([64, 2], f32, tag="fold")
        nc.sync.dma_start(fold[:], sq[64:128, :])
        loc = sm.tile([64, 2], f32, tag="loc")
        nc.vector.tensor_add(loc[:], sq[0:64, :], fold[:])
        # all-reduce across cores
        cin = dram.tile([64, 2], f32)
        cout = dram.tile([64, 2], f32)
        if for_sim:
            # emulate 8 identical cores so single-core sim matches host math
            nc.vector.tensor_scalar_mul(loc[:], loc[:], float(N_CORES))
        nc.sync.dma_start(cin[:], loc[:])
        if for_sim:
            nc.sync.dma_start(cout[:], cin[:])
        else:
            nc.gpsimd.collective_compute(
                "AllReduce", mybir.AluOpType.add,
                replica_groups=[list(range(N_CORES))],
                ins=[cin.opt()], outs=[cout.opt()],
            )
        g = sm.tile([64, 2], f32, tag="g")
        nc.sync.dma_start(g[:], cout[:])
        # mean/var -> a = gamma/sqrt(var+eps), b = beta - mean*a
        mean = sm.tile([64, 1], f32, tag="mean")
        e2 = sm.tile([64, 1], f32, tag="e2")
        nc.vector.tensor_scalar_mul(mean[:], g[:, 0:1], 1.0 / TOT)
        nc.vector.tensor_scalar_mul(e2[:], g[:, 1:2], 1.0 / TOT)
        msq = sm.tile([64, 1], f32, tag="msq")
        nc.vector.tensor_mul(msq[:], mean[:], mean[:])
        vpe = sm.tile([64, 1], f32, tag="vpe")
        nc.vector.tensor_sub(vpe[:], e2[:], msq[:])
        nc.vector.tensor_scalar_add(vpe[:], vpe[:], EPS)
        sqv = sm.tile([64, 1], f32, tag="sqv")
        nc.scalar.sqrt(sqv[:], vpe[:])
        inv = sm.tile([64, 1], f32, tag="inv")
        nc.vector.reciprocal(inv[:], sqv[:])
        # one Newton step: inv *= 1.5 - 0.5*vpe*inv*inv
        t1 = sm.tile([64, 1], f32, tag="t1")
        nc.vector.tensor_mul(t1[:], inv[:], inv[:])
        nc.vector.tensor_mul(t1[:], t1[:], vpe[:])
        nc.vector.tensor_scalar(t1[:], t1[:], -0.5, 1.5,
                                mybir.AluOpType.mult, mybir.AluOpType.add)
        nc.vector.tensor_mul(inv[:], inv[:], t1[:])
        ab = sm.tile([128, 2], f32, tag="ab")
        nc.vector.tensor_mul(ab[0:64, 0:1], inv[:], gbt[:, 0:1])
        t2 = sm.tile([64, 1], f32, tag="t2")
        nc.vector.tensor_mul(t2[:], mean[:], ab[0:64, 0:1])
        nc.vector.tensor_sub(ab[0:64, 1:2], gbt[:, 1:2], t2[:])
        nc.sync.dma_start(ab[64:128, :], ab[0:64, :])

        # normalize + relu in chunks, then stream out
        PCH = 7            # pairs per chunk
        CHUNK = PCH * NPT  # 3136
        for c in range(NPAIR // PCH):
            ysl = yt[:, c * CHUNK:(c + 1) * CHUNK]
            nc.scalar.activation(ysl, ysl, mybir.ActivationFunctionType.Relu,
                                 bias=ab[:, 1:2], scale=ab[:, 0:1])
            for m in range(c * PCH, (c + 1) * PCH):
                yv = yt[:, m * NPT:(m + 1) * NPT]
                for colh in range(2):
                    g = 2 * m + colh
                    t_g, i_g = g // TPT, g % TPT
                    h0 = i_g * HB
                    src2 = yv[colh * 64:(colh + 1) * 64, :].rearrange(
                        "p (r w) -> p r w", r=HB, w=W)
                    nc.sync.dma_start(out_d[:, t_g, h0:h0 + HB, :], src2[:])
    nc.compile()
    return nc


_NC = None
_last_in_maps = None


def _get_nc():
    global _NC
    if _NC is None:
        _NC = _build()
    return _NC


def kernel(x, weights, biases, W1, b1, W2, b2, gamma, beta, epochs_num):
    x = np.asarray(x, np.float32)
    weights = np.asarray(weights, np.float32)
    biases = np.asarray(biases, np.float32)
    W1 = np.asarray(W1, np.float32)
    b1 = np.asarray(b1, np.float32)
    W2 = np.asarray(W2, np.float32)
    b2 = np.asarray(b2, np.float32)
    gamma = np.asarray(gamma, np.float32)
    beta = np.asarray(beta, np.float32)
    ep = int(epochs_num)

    # host gating (matches reference math in fp32)
    tau = np.float32(30.0 - 2.9 * ep) if ep < 10 else np.float32(1.0)
    pooled = x.max(axis=(2, 3, 4))                    # (B, C_IN)
    h = np.maximum(pooled @ W1.T + b1, 0.0)
    logits = h @ W2.T + b2
    z = logits / tau
    z = z - z.max(axis=1, keepdims=True)
    ez = np.exp(z)
    phi = (ez / ez.sum(axis=1, keepdims=True)).astype(np.float32)  # (B, NW)

    dyn_w = np.einsum("bn,noidhw->boidhw", phi, weights).astype(np.float32)
    dyn_b = (phi @ biases).astype(np.float32)          # (B, DIMS)

    gb = np.stack([gamma, beta], axis=1).astype(np.float32)

    in_maps = []
    for b in range(B):
        wpk = np.zeros((128, NSLOT_A * 64), np.float32)
        for i, (dt, dh, dw) in enumerate(A_TAPS):
            wpk[0:64, i * 64:(i + 1) * 64] = dyn_w[b, :, :, dt, dh, dw].T
        for i, (dt, dh, dw) in enumerate(B_TAPS):
            wpk[64:128, i * 64:(i + 1) * 64] = dyn_w[b, :, :, dt, dh, dw].T
        in_maps.append({
            "xbf": x[b].astype(ml_dtypes.bfloat16),
            "wpk": wpk.astype(ml_dtypes.bfloat16),
            "dynb": np.tile(dyn_b[b][:, None], (2, 1)).astype(np.float32),
            "gb": gb,
        })

    global _last_in_maps
    _last_in_maps = in_maps
    nc = _get_nc()
    res = run_bass_kernel_spmd(nc, in_maps, core_ids=list(range(N_CORES)))
    out = np.stack([res.results[b]["out"] for b in range(B)], axis=0)
    return out.astype(np.float32)
